# revision 40
# baseline (speedup 1.0000x reference)
"""DGCNN layer (knn graph -> edge MLP -> mean aggregation) on 8 trn2 cores.

Self-contained: hardcodes shapes N=16384, C=64, O=128, K=16 and the
data-parallel-over-nodes sharding (2048 rows per core, x replicated).

v3.5: scheduling rework of the v2 fp16-score kernel; 765us HW (from
1442us).  The v2 trace showed the DVE idle ~17.6us at every tile
boundary: the gpsimd queue saturates (16 indirect gathers at ~1.4us
each -- multi-offset and compute_op gathers do NOT work on real HW --
plus the u-add/tree chain), stage2 slides to the boundary, and the
Tile scheduler places each out_stage transpose directly behind its
tree deps, stalling the in-order PE queue before the next tile's
distance matmuls.  Changes vs v2:
  - u-add moved to the DVE so the gpsimd queue stays ~74% busy
    (16 gathers + halving tree only); stage2 now completes mid-scan
    and the transposes never block the PE.  DVE is 100% busy.
  - V table / u' prep issued AFTER tile 0's scan (warmup 55us -> 8us);
    V streamed to DRAM in 4 double-buffered chunks.
  - stage2 issued at depth 3, out_stages drained after the main loop
    (the scheduler repositions them next to their deps anyway).
Top-k math identical to v2 (fp16 scores + hi/lo norm rows, exact
top-16 via per-1024-group max8/find_index8 + threshold trick).
"""

import numpy as np

N, C, O, K = 16384, 64, 128, 16
NCORES = 8
RLOC = N // NCORES          # 2048 rows per core
NT = RLOC // 128            # 16 row-tiles per core
GRP = 1024                  # top-k group width (exactness verified offline)
NG = N // GRP               # 16 groups per row
CAUG = C + 2                # fp16 contraction rows: 64 x + sq_hi + sq_lo
NEG = -3.0e38
VCHUNKS = 4                 # V table DRAM write, chunked
VROWS = N // VCHUNKS        # 4096 rows per chunk

_CACHE = {}


def _build_module():
    import concourse.bass as bass
    import concourse.bacc as bacc
    import concourse.mybir as mybir
    from concourse.tile import TileContext
    from concourse.masks import make_identity

    fp32 = mybir.dt.float32
    f16 = mybir.dt.float16
    u32 = mybir.dt.uint32
    Alu = mybir.AluOpType
    Act = mybir.ActivationFunctionType

    nc = bacc.Bacc()
    XTAH = nc.dram_tensor("XTAH", [CAUG, N], f16, kind="ExternalInput")
    XLH = nc.dram_tensor("XLH", [CAUG, RLOC], f16, kind="ExternalInput")
    xT = nc.dram_tensor("xT", [C, N], fp32, kind="ExternalInput")
    xlocT = nc.dram_tensor("xlocT", [C, RLOC], fp32, kind="ExternalInput")
    W1 = nc.dram_tensor("W1", [2 * C, O], fp32, kind="ExternalInput")
    b1 = nc.dram_tensor("b1", [1, O], fp32, kind="ExternalInput")
    W2 = nc.dram_tensor("W2", [O, O], fp32, kind="ExternalInput")
    b2 = nc.dram_tensor("b2", [1, O], fp32, kind="ExternalInput")
    out = nc.dram_tensor("out", [RLOC, O], fp32, kind="ExternalOutput")
    Vd = nc.dram_tensor("Vdram", [N, O], f16)  # internal: V = x @ W1b (fp16)

    with TileContext(nc) as tc:
        with (
            tc.tile_pool(name="persist", bufs=1) as pp,
            tc.tile_pool(name="vchunk", bufs=2) as vcp,
            tc.tile_pool(name="s_ps", bufs=3, space="PSUM") as s_ps,
            tc.tile_pool(name="aux_ps", bufs=2, space="PSUM") as aux_ps,
            tc.tile_pool(name="atp", bufs=3) as atp,
            tc.tile_pool(name="sb", bufs=2) as sb,
            tc.tile_pool(name="sb2", bufs=2) as sb2,
        ):
            # persistent SBUF state
            XTa = pp.tile([CAUG, N], f16)          # fp16 x^T + sq hi/lo rows
            xloca = pp.tile([CAUG, RLOC], f16)     # fp16 xloc^T + ones rows
            W1a_t = pp.tile([C, O], fp32)
            W1b_t = pp.tile([C, O], fp32)
            W1d_t = pp.tile([C, O], fp32)          # W1a - W1b
            W2_t = pp.tile([O, O], fp32)
            b1_t = pp.tile([1, O], fp32)
            b2_t = pp.tile([1, O], fp32)
            ones_col = pp.tile([1, 128], fp32)     # lhsT for bias seeding
            ident = pp.tile([128, 128], fp32)
            idxoff = pp.tile([128, NG * 8], fp32)  # +1-offset global base per group
            Usb = pp.tile([128, NT * 128], f16)    # u' = xloc@(W1a-W1b)+b1, tile-major
            mtall = pp.tile([128, NT * O], fp32)   # per-tile k-sums, out-mm batched
            W1b16 = pp.tile([C, O], f16)
            W1d16 = pp.tile([C, O], f16)
            b1_16 = pp.tile([1, O], f16)
            ones16 = pp.tile([1, 128], f16)

            nc.sync.dma_start(out=xloca[:, :], in_=XLH[:, :])
            nc.sync.dma_start(out=W1a_t[:, :], in_=W1[0:C, :])
            nc.sync.dma_start(out=W1b_t[:, :], in_=W1[C : 2 * C, :])
            nc.sync.dma_start(out=W2_t[:, :], in_=W2[:, :])
            nc.sync.dma_start(out=b1_t[:, :], in_=b1[:, :])
            nc.sync.dma_start(out=b2_t[:, :], in_=b2[:, :])
            for ch in range(N // 2048):
                nc.sync.dma_start(
                    out=XTa[:, ch * 2048 : (ch + 1) * 2048],
                    in_=XTAH[:, ch * 2048 : (ch + 1) * 2048],
                )

            nc.vector.memset(ones_col[:, :], 1.0)
            for g in range(NG):
                nc.vector.memset(idxoff[:, g * 8 : (g + 1) * 8], float(g * GRP + 1))
            make_identity(nc, ident[:, :])
            nc.vector.tensor_tensor(
                out=W1d_t[:, :], in0=W1a_t[:, :], in1=W1b_t[:, :], op=Alu.subtract
            )
            nc.scalar.activation(out=W1b16[:, :], in_=W1b_t[:, :], func=Act.Copy)
            nc.scalar.activation(out=W1d16[:, :], in_=W1d_t[:, :], func=Act.Copy)
            nc.scalar.activation(out=b1_16[:, :], in_=b1_t[:, :], func=Act.Copy)
            nc.vector.memset(ones16[:, :], 1.0)

            # ---- prep pieces (issued AFTER the first two tiles' scans) ----
            def v_chunk(c):
                # V rows [c*4096, (c+1)*4096) = x @ W1b -> DRAM (row-major)
                Vsb = vcp.tile([128, VROWS // 128 * 128], f16, tag="vsb")
                for b in range(VROWS // 128):
                    col = c * (VROWS // 128) + b
                    vp = aux_ps.tile([128, O], fp32, tag="aux")
                    nc.tensor.matmul(
                        out=vp[:, :],
                        lhsT=XTa[0:C, col * 128 : (col + 1) * 128],
                        rhs=W1b16[:, :],
                        start=True,
                        stop=True,
                    )
                    nc.scalar.activation(
                        out=Vsb[:, b * 128 : (b + 1) * 128],
                        in_=vp[:, :],
                        func=Act.Copy,
                    )
                nc.sync.dma_start(
                    out=Vd[c * VROWS : (c + 1) * VROWS, :].rearrange(
                        "(b p) f -> p b f", p=128
                    ),
                    in_=Vsb[:, :].rearrange("p (b f) -> p b f", f=128),
                )

            def u_prep():
                # u' = xloc @ (W1a - W1b) + b1  (tile-major into Usb)
                for t in range(NT):
                    up = aux_ps.tile([128, O], fp32, tag="aux")
                    nc.tensor.matmul(
                        out=up[:, :],
                        lhsT=ones16[:, :],
                        rhs=b1_16[:, :],
                        start=True,
                        stop=False,
                    )
                    nc.tensor.matmul(
                        out=up[:, :],
                        lhsT=xloca[0:C, t * 128 : (t + 1) * 128],
                        rhs=W1d16[:, :],
                        start=False,
                        stop=True,
                    )
                    nc.scalar.activation(
                        out=Usb[:, t * 128 : (t + 1) * 128],
                        in_=up[:, :],
                        func=Act.Copy,
                    )  # Usb is fp16: enables the 2x DVE u-add

            # ---- main stages ----
            ND = NG  # scan groups per row-tile (1024-wide, straight from PSUM)
            def stage1(t, cbs=None):
                # distance matmuls, exact top-16 indices.  (An ACT-staged
                # 2048-wide SBUF scan variant saved ~78us of DVE time but
                # cost ~220us of boundary stalls -- the fp32 PSUM->SBUF ACT
                # copies get no accel and sit in the DVE's feed path.)
                lhsT_t = xloca[:, t * 128 : (t + 1) * 128]
                cand = sb2.tile([128, ND * 8], fp32, tag="cand")
                cidx = sb2.tile([128, ND * 8], u32, tag="cidx")
                for g in range(NG):
                    sp = s_ps.tile([128, GRP], fp32, tag="sp")
                    for q in range(GRP // 512):
                        nc.tensor.matmul(
                            out=sp[:, q * 512 : (q + 1) * 512],
                            lhsT=lhsT_t,
                            rhs=XTa[:, g * GRP + q * 512 : g * GRP + (q + 1) * 512],
                            start=True,
                            stop=True,
                        )
                    # max8 + find_index8 straight from PSUM
                    nc.vector.max(out=cand[:, g * 8 : (g + 1) * 8], in_=sp[:, :])
                    nc.vector.max_index(
                        out=cidx[:, g * 8 : (g + 1) * 8],
                        in_max=cand[:, g * 8 : (g + 1) * 8],
                        in_values=sp[:, :],
                    )

                # phase B: exact top-16 index extraction (64 candidates)
                cidx_f = sb2.tile([128, ND * 8], fp32, tag="cidx_f")
                nc.vector.tensor_copy(out=cidx_f[:, :], in_=cidx[:, :])
                idxp1 = sb2.tile([128, ND * 8], fp32, tag="idxp1")
                nc.vector.tensor_tensor(
                    out=idxp1[:, :], in0=cidx_f[:, :], in1=idxoff[:, :], op=Alu.add
                )
                m1 = sb2.tile([128, 8], fp32, tag="m1")
                nc.vector.max(out=m1[:, :], in_=cand[:, :])
                cand2 = sb2.tile([128, ND * 8], fp32, tag="cand2")
                nc.vector.match_replace(
                    out=cand2[:, :], in_to_replace=m1[:, :], in_values=cand[:, :],
                    imm_value=NEG,
                )
                m2 = sb2.tile([128, 8], fp32, tag="m2")
                nc.vector.max(out=m2[:, :], in_=cand2[:, :])
                mask = sb2.tile([128, ND * 8], fp32, tag="mask")
                nc.vector.tensor_tensor(
                    out=mask[:, :],
                    in0=cand[:, :],
                    in1=m2[:, 7:8].to_broadcast([128, ND * 8]),
                    op=Alu.is_ge,
                )
                midx = sb2.tile([128, ND * 8], fp32, tag="midx")
                nc.vector.tensor_tensor(
                    out=midx[:, :], in0=mask[:, :], in1=idxp1[:, :], op=Alu.mult
                )
                winners = sb2.tile([128, 16], fp32, tag="winners")
                nc.vector.max(out=winners[:, 0:8], in_=midx[:, :])
                midx2 = sb2.tile([128, ND * 8], fp32, tag="midx2")
                nc.vector.match_replace(
                    out=midx2[:, :], in_to_replace=winners[:, 0:8],
                    in_values=midx[:, :], imm_value=0.0,
                )
                nc.vector.max(out=winners[:, 8:16], in_=midx2[:, :])
                nc.vector.tensor_scalar_add(winners[:, :], winners[:, :], -1.0)
                idxu = sb2.tile([128, 16], u32, tag="idxu")
                nc.vector.tensor_copy(out=idxu[:, :], in_=winners[:, :])
                return idxu

            def gather_stage(t, idxu):
                # 16 single-offset indirect DMAs (HW ignores all but the
                # first offset column of a multi-offset AP -- verified on HW)
                Gt = atp.tile([128, K * O], f16, tag="at")
                for k in range(K):
                    nc.gpsimd.indirect_dma_start(
                        out=Gt[:, k * O : (k + 1) * O],
                        out_offset=None,
                        in_=Vd[:, :],
                        in_offset=bass.IndirectOffsetOnAxis(
                            ap=idxu[:, k : k + 1], axis=0
                        ),
                    )
                return Gt

            def stage2(t, Gt):
                # a = G + u' ; h = relu(a) ; m-sum over k via halving tree
                # u-add runs on the DVE: the gpsimd queue saturates with the
                # 16 gathers/tile, sliding stage2 to the tile boundary where
                # the scheduler-placed transposes then stall the PE queue.
                # All-fp16 operands put the add in the 2x_1p DVE mode.
                At = sb.tile([128, K * O], f16, tag="at2")
                nc.vector.tensor_tensor(
                    out=At[:, :].rearrange("p (k f) -> p k f", k=K),
                    in0=Gt[:, :].rearrange("p (k f) -> p k f", k=K),
                    in1=Usb[:, t * 128 : (t + 1) * 128]
                    .rearrange("p (k f) -> p k f", k=1)
                    .to_broadcast([128, K, O]),
                    op=Alu.add,
                )
                Ht = sb.tile([128, K * O], fp32, tag="ht")
                nc.scalar.activation(out=Ht[:, :], in_=At[:, :], func=Act.Relu)
                T1 = sb2.tile([128, 8 * O], fp32, tag="T1")
                nc.gpsimd.tensor_tensor(
                    out=T1[:, :], in0=Ht[:, 0 : 8 * O], in1=Ht[:, 8 * O : 16 * O],
                    op=Alu.add,
                )
                T2 = sb2.tile([128, 4 * O], fp32, tag="T2")
                nc.gpsimd.tensor_tensor(
                    out=T2[:, :], in0=T1[:, 0 : 4 * O], in1=T1[:, 4 * O : 8 * O],
                    op=Alu.add,
                )
                T3 = sb2.tile([128, 2 * O], fp32, tag="T3")
                nc.gpsimd.tensor_tensor(
                    out=T3[:, :], in0=T2[:, 0 : 2 * O], in1=T2[:, 2 * O : 4 * O],
                    op=Alu.add,
                )
                nc.gpsimd.tensor_tensor(
                    out=mtall[:, t * O : (t + 1) * O],
                    in0=T3[:, 0:O], in1=T3[:, O : 2 * O], op=Alu.add,
                )

            def out_stage(t):
                # out_tile = (m/16) @ W2 + b2
                mtp = aux_ps.tile([128, 128], fp32, tag="aux")
                nc.tensor.transpose(
                    out=mtp[:, :], in_=mtall[:, t * O : (t + 1) * O],
                    identity=ident[:, :],
                )
                mT = sb2.tile([128, 128], fp32, tag="mT")
                nc.scalar.activation(
                    out=mT[:, :], in_=mtp[:, :], func=Act.Copy, scale=1.0 / K
                )
                op_ = aux_ps.tile([128, O], fp32, tag="aux")
                nc.tensor.matmul(
                    out=op_[:, :], lhsT=ones_col[:, :], rhs=b2_t[:, :],
                    start=True, stop=False,
                )
                nc.tensor.matmul(
                    out=op_[:, :], lhsT=mT[:, :], rhs=W2_t[:, :],
                    start=False, stop=True,
                )
                osb = sb2.tile([128, O], fp32, tag="osb")
                nc.scalar.activation(out=osb[:, :], in_=op_[:, :], func=Act.Copy)
                nc.sync.dma_start(
                    out=out[t * 128 : (t + 1) * 128, :], in_=osb[:, :]
                )

            # pipeline:
            #   t=0: pure scan (PE starts distance MMs immediately),
            #        then ALL V/u' prep (program order: every Vd write and
            #        the Usb fill must precede the first gather/prefill)
            #   t>=2: gather(t-2) at top of iteration t
            #   stage2(tg) at g==8 once two gathers are in flight
            #   out_stage(t-5) at g==13 (deep delay so the PE queue never
            #   holds a transpose whose gpsimd input chain is still cold)
            # The gpsimd queue is nearly saturated (16 gathers ~22.5us +
            # stage2 ~11us per ~37us scan), so stage2 execution slides to
            # the tile boundaries.  Nothing may couple the PE queue to it:
            # ALL out_stages (transpose + W2 matmuls) are issued after the
            # main loop, where their tree inputs are long complete.
            pend = []      # (t, idxu) awaiting gather issue
            done_g = []    # (t, Gt) awaiting stage2
            out_pend = []  # tiles whose stage2 is issued, awaiting out_stage
            for t in range(NT):
                if t >= 1 and pend:
                    tg, ixu = pend.pop(0)
                    done_g.append((tg, gather_stage(tg, ixu)))
                if len(done_g) >= 2:
                    tg2, Gt2 = done_g.pop(0)
                    stage2(tg2, Gt2)
                    out_pend.append(tg2)
                pend.append((t, stage1(t)))
                if t == 0:
                    u_prep()
                    for c in range(VCHUNKS):
                        v_chunk(c)
            # drain: stage2s whose gathers completed during the last scans
            # go FIRST, so the final tile's 16 gathers (~22us of gpsimd)
            # overlap the tail MLP chains instead of blocking them
            while len(done_g) > 0 and pend:
                tg2, Gt2 = done_g.pop(0)
                stage2(tg2, Gt2)
                out_pend.append(tg2)
                tg, ixu = pend.pop(0)
                done_g.append((tg, gather_stage(tg, ixu)))
            while done_g:
                tg2, Gt2 = done_g.pop(0)
                stage2(tg2, Gt2)
                out_pend.append(tg2)
            while out_pend:
                out_stage(out_pend.pop(0))
    nc.finalize()
    return nc


LAST_RESULTS = None


def kernel(x, W1, b1, W2, b2):
    global LAST_RESULTS
    from concourse.bass_utils import run_bass_kernel_spmd

    if "nc" not in _CACHE:
        _CACHE["nc"] = _build_module()
    nc = _CACHE["nc"]

    x = np.ascontiguousarray(np.asarray(x, dtype=np.float32))
    xh = x.astype(np.float16)
    v = -0.5 * (xh.astype(np.float32) ** 2).sum(axis=1)
    hi = v.astype(np.float16)
    lo = (v - hi.astype(np.float32)).astype(np.float16)
    XTAH = np.zeros((CAUG, N), dtype=np.float16)
    XTAH[0:C, :] = xh.T
    XTAH[C, :] = hi
    XTAH[C + 1, :] = lo
    XLH_full = np.zeros((CAUG, N), dtype=np.float16)
    XLH_full[0:C, :] = xh.T
    XLH_full[C, :] = 1.0
    XLH_full[C + 1, :] = 1.0

    xT = np.ascontiguousarray(x.T)
    in_maps = []
    for c in range(NCORES):
        sl = slice(c * RLOC, (c + 1) * RLOC)
        in_maps.append(
            {
                "XTAH": XTAH,
                "XLH": np.ascontiguousarray(XLH_full[:, sl]),
                "xT": xT,
                "xlocT": np.ascontiguousarray(x[sl, :].T),
                "W1": np.ascontiguousarray(np.asarray(W1, dtype=np.float32)),
                "b1": np.ascontiguousarray(
                    np.asarray(b1, dtype=np.float32).reshape(1, O)
                ),
                "W2": np.ascontiguousarray(np.asarray(W2, dtype=np.float32)),
                "b2": np.ascontiguousarray(
                    np.asarray(b2, dtype=np.float32).reshape(1, O)
                ),
            }
        )
    import os

    res = run_bass_kernel_spmd(
        nc,
        in_maps,
        core_ids=list(range(NCORES)),
        trace=bool(int(os.environ.get("KERNEL_TRACE", "0"))),
    )
    LAST_RESULTS = res
    outs = [res.results[c]["out"] for c in range(NCORES)]
    return np.concatenate(outs, axis=0).astype(np.float32)


# revision 41
# speedup vs baseline: 1.0057x; 1.0057x over previous
"""DGCNN layer (knn graph -> edge MLP -> mean aggregation) on 8 trn2 cores.

Self-contained: hardcodes shapes N=16384, C=64, O=128, K=16 and the
data-parallel-over-nodes sharding (2048 rows per core, x replicated).

v3.5: scheduling rework of the v2 fp16-score kernel; 765us HW (from
1442us).  The v2 trace showed the DVE idle ~17.6us at every tile
boundary: the gpsimd queue saturates (16 indirect gathers at ~1.4us
each -- multi-offset and compute_op gathers do NOT work on real HW --
plus the u-add/tree chain), stage2 slides to the boundary, and the
Tile scheduler places each out_stage transpose directly behind its
tree deps, stalling the in-order PE queue before the next tile's
distance matmuls.  Changes vs v2:
  - u-add moved to the DVE so the gpsimd queue stays ~74% busy
    (16 gathers + halving tree only); stage2 now completes mid-scan
    and the transposes never block the PE.  DVE is 100% busy.
  - V table / u' prep issued AFTER tile 0's scan (warmup 55us -> 8us);
    V streamed to DRAM in 4 double-buffered chunks.
  - stage2 issued at depth 3, out_stages drained after the main loop
    (the scheduler repositions them next to their deps anyway).
Top-k math identical to v2 (fp16 scores + hi/lo norm rows, exact
top-16 via per-1024-group max8/find_index8 + threshold trick).
"""

import numpy as np

N, C, O, K = 16384, 64, 128, 16
NCORES = 8
RLOC = N // NCORES          # 2048 rows per core
NT = RLOC // 128            # 16 row-tiles per core
GRP = 1024                  # top-k group width (exactness verified offline)
NG = N // GRP               # 16 groups per row
CAUG = C + 2                # fp16 contraction rows: 64 x + sq_hi + sq_lo
NEG = -3.0e38
VCHUNKS = 4                 # V table DRAM write, chunked
VROWS = N // VCHUNKS        # 4096 rows per chunk

_CACHE = {}


def _build_module():
    import concourse.bass as bass
    import concourse.bacc as bacc
    import concourse.mybir as mybir
    from concourse.tile import TileContext
    from concourse.masks import make_identity

    fp32 = mybir.dt.float32
    f16 = mybir.dt.float16
    u32 = mybir.dt.uint32
    Alu = mybir.AluOpType
    Act = mybir.ActivationFunctionType

    nc = bacc.Bacc()
    XTAH = nc.dram_tensor("XTAH", [CAUG, N], f16, kind="ExternalInput")
    XLH = nc.dram_tensor("XLH", [CAUG, RLOC], f16, kind="ExternalInput")
    xT = nc.dram_tensor("xT", [C, N], fp32, kind="ExternalInput")
    xlocT = nc.dram_tensor("xlocT", [C, RLOC], fp32, kind="ExternalInput")
    W1 = nc.dram_tensor("W1", [2 * C, O], fp32, kind="ExternalInput")
    b1 = nc.dram_tensor("b1", [1, O], fp32, kind="ExternalInput")
    W2 = nc.dram_tensor("W2", [O, O], fp32, kind="ExternalInput")
    b2 = nc.dram_tensor("b2", [1, O], fp32, kind="ExternalInput")
    out = nc.dram_tensor("out", [RLOC, O], fp32, kind="ExternalOutput")
    Vd = nc.dram_tensor("Vdram", [N, O], f16)  # internal: V = x @ W1b (fp16)

    with TileContext(nc) as tc:
        with (
            tc.tile_pool(name="persist", bufs=1) as pp,
            tc.tile_pool(name="vchunk", bufs=2) as vcp,
            tc.tile_pool(name="s_ps", bufs=3, space="PSUM") as s_ps,
            tc.tile_pool(name="aux_ps", bufs=2, space="PSUM") as aux_ps,
            tc.tile_pool(name="atp", bufs=3) as atp,
            tc.tile_pool(name="sb", bufs=2) as sb,
            tc.tile_pool(name="sb2", bufs=2) as sb2,
        ):
            # persistent SBUF state
            XTa = pp.tile([CAUG, N], f16)          # fp16 x^T + sq hi/lo rows
            xloca = pp.tile([CAUG, RLOC], f16)     # fp16 xloc^T + ones rows
            W1a_t = pp.tile([C, O], fp32)
            W1b_t = pp.tile([C, O], fp32)
            W1d_t = pp.tile([C, O], fp32)          # W1a - W1b
            W2_t = pp.tile([O, O], fp32)
            b1_t = pp.tile([1, O], fp32)
            b2_t = pp.tile([1, O], fp32)
            ones_col = pp.tile([1, 128], fp32)     # lhsT for bias seeding
            ident = pp.tile([128, 128], fp32)
            idxoff = pp.tile([128, NG * 8], fp32)  # +1-offset global base per group
            Usb = pp.tile([128, NT * 128], f16)    # u' = xloc@(W1a-W1b)+b1, tile-major
            mtall = pp.tile([128, NT * O], fp32)   # per-tile k-sums, out-mm batched
            W1b16 = pp.tile([C, O], f16)
            W1d16 = pp.tile([C, O], f16)
            b1_16 = pp.tile([1, O], f16)
            ones16 = pp.tile([1, 128], f16)

            nc.sync.dma_start(out=xloca[:, :], in_=XLH[:, :])
            nc.sync.dma_start(out=W1a_t[:, :], in_=W1[0:C, :])
            nc.sync.dma_start(out=W1b_t[:, :], in_=W1[C : 2 * C, :])
            nc.sync.dma_start(out=W2_t[:, :], in_=W2[:, :])
            nc.sync.dma_start(out=b1_t[:, :], in_=b1[:, :])
            nc.sync.dma_start(out=b2_t[:, :], in_=b2[:, :])
            for ch in range(N // 2048):
                nc.sync.dma_start(
                    out=XTa[:, ch * 2048 : (ch + 1) * 2048],
                    in_=XTAH[:, ch * 2048 : (ch + 1) * 2048],
                )

            nc.vector.memset(ones_col[:, :], 1.0)
            for g in range(NG):
                nc.vector.memset(idxoff[:, g * 8 : (g + 1) * 8], float(g * GRP + 1))
            make_identity(nc, ident[:, :])
            nc.vector.tensor_tensor(
                out=W1d_t[:, :], in0=W1a_t[:, :], in1=W1b_t[:, :], op=Alu.subtract
            )
            nc.scalar.activation(out=W1b16[:, :], in_=W1b_t[:, :], func=Act.Copy)
            nc.scalar.activation(out=W1d16[:, :], in_=W1d_t[:, :], func=Act.Copy)
            nc.scalar.activation(out=b1_16[:, :], in_=b1_t[:, :], func=Act.Copy)
            nc.vector.memset(ones16[:, :], 1.0)

            # ---- prep pieces (issued AFTER the first two tiles' scans) ----
            def v_chunk(c):
                # V rows [c*4096, (c+1)*4096) = x @ W1b -> DRAM (row-major)
                Vsb = vcp.tile([128, VROWS // 128 * 128], f16, tag="vsb")
                for b in range(VROWS // 128):
                    col = c * (VROWS // 128) + b
                    vp = aux_ps.tile([128, O], fp32, tag="aux")
                    nc.tensor.matmul(
                        out=vp[:, :],
                        lhsT=XTa[0:C, col * 128 : (col + 1) * 128],
                        rhs=W1b16[:, :],
                        start=True,
                        stop=True,
                    )
                    nc.scalar.activation(
                        out=Vsb[:, b * 128 : (b + 1) * 128],
                        in_=vp[:, :],
                        func=Act.Copy,
                    )
                nc.sync.dma_start(
                    out=Vd[c * VROWS : (c + 1) * VROWS, :].rearrange(
                        "(b p) f -> p b f", p=128
                    ),
                    in_=Vsb[:, :].rearrange("p (b f) -> p b f", f=128),
                )

            def u_prep():
                # u' = xloc @ (W1a - W1b) + b1  (tile-major into Usb)
                for t in range(NT):
                    up = aux_ps.tile([128, O], fp32, tag="aux")
                    nc.tensor.matmul(
                        out=up[:, :],
                        lhsT=ones16[:, :],
                        rhs=b1_16[:, :],
                        start=True,
                        stop=False,
                    )
                    nc.tensor.matmul(
                        out=up[:, :],
                        lhsT=xloca[0:C, t * 128 : (t + 1) * 128],
                        rhs=W1d16[:, :],
                        start=False,
                        stop=True,
                    )
                    nc.scalar.activation(
                        out=Usb[:, t * 128 : (t + 1) * 128],
                        in_=up[:, :],
                        func=Act.Copy,
                    )  # Usb is fp16: enables the 2x DVE u-add

            # ---- main stages ----
            ND = NG  # scan groups per row-tile (1024-wide, straight from PSUM)
            def stage1(t, cbs=None):
                # distance matmuls, exact top-16 indices.  (An ACT-staged
                # 2048-wide SBUF scan variant saved ~78us of DVE time but
                # cost ~220us of boundary stalls -- the fp32 PSUM->SBUF ACT
                # copies get no accel and sit in the DVE's feed path.)
                lhsT_t = xloca[:, t * 128 : (t + 1) * 128]
                cand = sb2.tile([128, ND * 8], fp32, tag="cand")
                cidx = sb2.tile([128, ND * 8], u32, tag="cidx")
                for g in range(NG):
                    sp = s_ps.tile([128, GRP], fp32, tag="sp")
                    for q in range(GRP // 512):
                        nc.tensor.matmul(
                            out=sp[:, q * 512 : (q + 1) * 512],
                            lhsT=lhsT_t,
                            rhs=XTa[:, g * GRP + q * 512 : g * GRP + (q + 1) * 512],
                            start=True,
                            stop=True,
                        )
                    # max8 + find_index8 straight from PSUM
                    nc.vector.max(out=cand[:, g * 8 : (g + 1) * 8], in_=sp[:, :])
                    nc.vector.max_index(
                        out=cidx[:, g * 8 : (g + 1) * 8],
                        in_max=cand[:, g * 8 : (g + 1) * 8],
                        in_values=sp[:, :],
                    )

                # phase B: exact top-16 index extraction (64 candidates)
                cidx_f = sb2.tile([128, ND * 8], fp32, tag="cidx_f")
                nc.vector.tensor_copy(out=cidx_f[:, :], in_=cidx[:, :])
                idxp1 = sb2.tile([128, ND * 8], fp32, tag="idxp1")
                nc.vector.tensor_tensor(
                    out=idxp1[:, :], in0=cidx_f[:, :], in1=idxoff[:, :], op=Alu.add
                )
                m1 = sb2.tile([128, 8], fp32, tag="m1")
                nc.vector.max(out=m1[:, :], in_=cand[:, :])
                cand2 = sb2.tile([128, ND * 8], fp32, tag="cand2")
                nc.vector.match_replace(
                    out=cand2[:, :], in_to_replace=m1[:, :], in_values=cand[:, :],
                    imm_value=NEG,
                )
                m2 = sb2.tile([128, 8], fp32, tag="m2")
                nc.vector.max(out=m2[:, :], in_=cand2[:, :])
                mask = sb2.tile([128, ND * 8], fp32, tag="mask")
                nc.vector.tensor_tensor(
                    out=mask[:, :],
                    in0=cand[:, :],
                    in1=m2[:, 7:8].to_broadcast([128, ND * 8]),
                    op=Alu.is_ge,
                )
                midx = sb2.tile([128, ND * 8], fp32, tag="midx")
                nc.vector.tensor_tensor(
                    out=midx[:, :], in0=mask[:, :], in1=idxp1[:, :], op=Alu.mult
                )
                winners = sb2.tile([128, 16], fp32, tag="winners")
                nc.vector.max(out=winners[:, 0:8], in_=midx[:, :])
                midx2 = sb2.tile([128, ND * 8], fp32, tag="midx2")
                nc.vector.match_replace(
                    out=midx2[:, :], in_to_replace=winners[:, 0:8],
                    in_values=midx[:, :], imm_value=0.0,
                )
                nc.vector.max(out=winners[:, 8:16], in_=midx2[:, :])
                nc.vector.tensor_scalar_add(winners[:, :], winners[:, :], -1.0)
                idxu = sb2.tile([128, 16], u32, tag="idxu")
                nc.vector.tensor_copy(out=idxu[:, :], in_=winners[:, :])
                return idxu

            def gather_stage(t, idxu):
                # 16 single-offset indirect DMAs (HW ignores all but the
                # first offset column of a multi-offset AP -- verified on HW)
                Gt = atp.tile([128, K * O], f16, tag="at")
                for k in range(K):
                    nc.gpsimd.indirect_dma_start(
                        out=Gt[:, k * O : (k + 1) * O],
                        out_offset=None,
                        in_=Vd[:, :],
                        in_offset=bass.IndirectOffsetOnAxis(
                            ap=idxu[:, k : k + 1], axis=0
                        ),
                    )
                return Gt

            def stage2(t, Gt):
                # a = G + u' ; h = relu(a) ; m-sum over k via halving tree
                # u-add runs on the DVE: the gpsimd queue saturates with the
                # 16 gathers/tile, sliding stage2 to the tile boundary where
                # the scheduler-placed transposes then stall the PE queue.
                # All-fp16 operands put the add in the 2x_1p DVE mode.
                At = sb.tile([128, K * O], f16, tag="at2")
                nc.vector.tensor_tensor(
                    out=At[:, :].rearrange("p (k f) -> p k f", k=K),
                    in0=Gt[:, :].rearrange("p (k f) -> p k f", k=K),
                    in1=Usb[:, t * 128 : (t + 1) * 128]
                    .rearrange("p (k f) -> p k f", k=1)
                    .to_broadcast([128, K, O]),
                    op=Alu.add,
                )
                Ht = sb.tile([128, K * O], fp32, tag="ht")
                nc.scalar.activation(out=Ht[:, :], in_=At[:, :], func=Act.Relu)
                T1 = sb2.tile([128, 8 * O], fp32, tag="T1")
                nc.gpsimd.tensor_tensor(
                    out=T1[:, :], in0=Ht[:, 0 : 8 * O], in1=Ht[:, 8 * O : 16 * O],
                    op=Alu.add,
                )
                T2 = sb2.tile([128, 4 * O], fp32, tag="T2")
                nc.gpsimd.tensor_tensor(
                    out=T2[:, :], in0=T1[:, 0 : 4 * O], in1=T1[:, 4 * O : 8 * O],
                    op=Alu.add,
                )
                T3 = sb2.tile([128, 2 * O], fp32, tag="T3")
                nc.gpsimd.tensor_tensor(
                    out=T3[:, :], in0=T2[:, 0 : 2 * O], in1=T2[:, 2 * O : 4 * O],
                    op=Alu.add,
                )
                nc.gpsimd.tensor_tensor(
                    out=mtall[:, t * O : (t + 1) * O],
                    in0=T3[:, 0:O], in1=T3[:, O : 2 * O], op=Alu.add,
                )

            def out_stage(t):
                # out_tile = (m/16) @ W2 + b2
                mtp = aux_ps.tile([128, 128], fp32, tag="aux")
                nc.tensor.transpose(
                    out=mtp[:, :], in_=mtall[:, t * O : (t + 1) * O],
                    identity=ident[:, :],
                )
                mT = sb2.tile([128, 128], fp32, tag="mT")
                nc.scalar.activation(
                    out=mT[:, :], in_=mtp[:, :], func=Act.Copy, scale=1.0 / K
                )
                op_ = aux_ps.tile([128, O], fp32, tag="aux")
                nc.tensor.matmul(
                    out=op_[:, :], lhsT=ones_col[:, :], rhs=b2_t[:, :],
                    start=True, stop=False,
                )
                nc.tensor.matmul(
                    out=op_[:, :], lhsT=mT[:, :], rhs=W2_t[:, :],
                    start=False, stop=True,
                )
                osb = sb2.tile([128, O], fp32, tag="osb")
                nc.scalar.activation(out=osb[:, :], in_=op_[:, :], func=Act.Copy)
                nc.sync.dma_start(
                    out=out[t * 128 : (t + 1) * 128, :], in_=osb[:, :]
                )

            # pipeline:
            #   t=0: pure scan (PE starts distance MMs immediately),
            #        then ALL V/u' prep (program order: every Vd write and
            #        the Usb fill must precede the first gather/prefill)
            #   t>=2: gather(t-2) at top of iteration t
            #   stage2(tg) at g==8 once two gathers are in flight
            #   out_stage(t-5) at g==13 (deep delay so the PE queue never
            #   holds a transpose whose gpsimd input chain is still cold)
            # The gpsimd queue is nearly saturated (16 gathers ~22.5us +
            # stage2 ~11us per ~37us scan), so stage2 execution slides to
            # the tile boundaries.  Nothing may couple the PE queue to it:
            # ALL out_stages (transpose + W2 matmuls) are issued after the
            # main loop, where their tree inputs are long complete.
            pend = []      # (t, idxu) awaiting gather issue
            done_g = []    # (t, Gt) awaiting stage2
            out_pend = []  # tiles whose stage2 is issued, awaiting out_stage
            for t in range(NT):
                if len(done_g) >= 2:
                    tg2, Gt2 = done_g.pop(0)
                    stage2(tg2, Gt2)
                    out_pend.append(tg2)
                if t >= 1 and pend:
                    tg, ixu = pend.pop(0)
                    done_g.append((tg, gather_stage(tg, ixu)))
                pend.append((t, stage1(t)))
                if t == 0:
                    u_prep()
                    for c in range(VCHUNKS):
                        v_chunk(c)
            while pend:
                tg, ixu = pend.pop(0)
                done_g.append((tg, gather_stage(tg, ixu)))
            while done_g:
                tg, Gt2 = done_g.pop(0)
                stage2(tg, Gt2)
                out_pend.append(tg)
            while out_pend:
                out_stage(out_pend.pop(0))
    nc.finalize()
    return nc


LAST_RESULTS = None


def kernel(x, W1, b1, W2, b2):
    global LAST_RESULTS
    from concourse.bass_utils import run_bass_kernel_spmd

    if "nc" not in _CACHE:
        _CACHE["nc"] = _build_module()
    nc = _CACHE["nc"]

    x = np.ascontiguousarray(np.asarray(x, dtype=np.float32))
    xh = x.astype(np.float16)
    v = -0.5 * (xh.astype(np.float32) ** 2).sum(axis=1)
    hi = v.astype(np.float16)
    lo = (v - hi.astype(np.float32)).astype(np.float16)
    XTAH = np.zeros((CAUG, N), dtype=np.float16)
    XTAH[0:C, :] = xh.T
    XTAH[C, :] = hi
    XTAH[C + 1, :] = lo
    XLH_full = np.zeros((CAUG, N), dtype=np.float16)
    XLH_full[0:C, :] = xh.T
    XLH_full[C, :] = 1.0
    XLH_full[C + 1, :] = 1.0

    xT = np.ascontiguousarray(x.T)
    in_maps = []
    for c in range(NCORES):
        sl = slice(c * RLOC, (c + 1) * RLOC)
        in_maps.append(
            {
                "XTAH": XTAH,
                "XLH": np.ascontiguousarray(XLH_full[:, sl]),
                "xT": xT,
                "xlocT": np.ascontiguousarray(x[sl, :].T),
                "W1": np.ascontiguousarray(np.asarray(W1, dtype=np.float32)),
                "b1": np.ascontiguousarray(
                    np.asarray(b1, dtype=np.float32).reshape(1, O)
                ),
                "W2": np.ascontiguousarray(np.asarray(W2, dtype=np.float32)),
                "b2": np.ascontiguousarray(
                    np.asarray(b2, dtype=np.float32).reshape(1, O)
                ),
            }
        )
    import os

    res = run_bass_kernel_spmd(
        nc,
        in_maps,
        core_ids=list(range(NCORES)),
        trace=bool(int(os.environ.get("KERNEL_TRACE", "0"))),
    )
    LAST_RESULTS = res
    outs = [res.results[c]["out"] for c in range(NCORES)]
    return np.concatenate(outs, axis=0).astype(np.float32)


# revision 42
# speedup vs baseline: 1.0062x; 1.0005x over previous
"""DGCNN layer (knn graph -> edge MLP -> mean aggregation) on 8 trn2 cores.

Self-contained: hardcodes shapes N=16384, C=64, O=128, K=16 and the
data-parallel-over-nodes sharding (2048 rows per core, x replicated).

v3.5: scheduling rework of the v2 fp16-score kernel; 765us HW (from
1442us).  The v2 trace showed the DVE idle ~17.6us at every tile
boundary: the gpsimd queue saturates (16 indirect gathers at ~1.4us
each -- multi-offset and compute_op gathers do NOT work on real HW --
plus the u-add/tree chain), stage2 slides to the boundary, and the
Tile scheduler places each out_stage transpose directly behind its
tree deps, stalling the in-order PE queue before the next tile's
distance matmuls.  Changes vs v2:
  - u-add moved to the DVE so the gpsimd queue stays ~74% busy
    (16 gathers + halving tree only); stage2 now completes mid-scan
    and the transposes never block the PE.  DVE is 100% busy.
  - V table / u' prep issued AFTER tile 0's scan (warmup 55us -> 8us);
    V streamed to DRAM in 4 double-buffered chunks.
  - stage2 issued at depth 3, out_stages drained after the main loop
    (the scheduler repositions them next to their deps anyway).
Top-k math identical to v2 (fp16 scores + hi/lo norm rows, exact
top-16 via per-1024-group max8/find_index8 + threshold trick).
"""

import numpy as np

N, C, O, K = 16384, 64, 128, 16
NCORES = 8
RLOC = N // NCORES          # 2048 rows per core
NT = RLOC // 128            # 16 row-tiles per core
GRP = 1024                  # top-k group width (exactness verified offline)
NG = N // GRP               # 16 groups per row
CAUG = C + 2                # fp16 contraction rows: 64 x + sq_hi + sq_lo
NEG = -3.0e38
VCHUNKS = 4                 # V table DRAM write, chunked
VROWS = N // VCHUNKS        # 4096 rows per chunk

_CACHE = {}


def _build_module():
    import concourse.bass as bass
    import concourse.bacc as bacc
    import concourse.mybir as mybir
    from concourse.tile import TileContext
    from concourse.masks import make_identity

    fp32 = mybir.dt.float32
    f16 = mybir.dt.float16
    u32 = mybir.dt.uint32
    Alu = mybir.AluOpType
    Act = mybir.ActivationFunctionType

    nc = bacc.Bacc()
    XTAH = nc.dram_tensor("XTAH", [CAUG, N], f16, kind="ExternalInput")
    XLH = nc.dram_tensor("XLH", [CAUG, RLOC], f16, kind="ExternalInput")
    xT = nc.dram_tensor("xT", [C, N], fp32, kind="ExternalInput")
    xlocT = nc.dram_tensor("xlocT", [C, RLOC], fp32, kind="ExternalInput")
    W1 = nc.dram_tensor("W1", [2 * C, O], fp32, kind="ExternalInput")
    b1 = nc.dram_tensor("b1", [1, O], fp32, kind="ExternalInput")
    W2 = nc.dram_tensor("W2", [O, O], fp32, kind="ExternalInput")
    b2 = nc.dram_tensor("b2", [1, O], fp32, kind="ExternalInput")
    out = nc.dram_tensor("out", [RLOC, O], fp32, kind="ExternalOutput")
    Vd = nc.dram_tensor("Vdram", [N, O], f16)  # internal: V = x @ W1b (fp16)

    with TileContext(nc) as tc:
        with (
            tc.tile_pool(name="persist", bufs=1) as pp,
            tc.tile_pool(name="vchunk", bufs=2) as vcp,
            tc.tile_pool(name="s_ps", bufs=3, space="PSUM") as s_ps,
            tc.tile_pool(name="aux_ps", bufs=2, space="PSUM") as aux_ps,
            tc.tile_pool(name="atp", bufs=3) as atp,
            tc.tile_pool(name="sb", bufs=2) as sb,
            tc.tile_pool(name="sb2", bufs=2) as sb2,
        ):
            # persistent SBUF state
            XTa = pp.tile([CAUG, N], f16)          # fp16 x^T + sq hi/lo rows
            xloca = pp.tile([CAUG, RLOC], f16)     # fp16 xloc^T + ones rows
            W1a_t = pp.tile([C, O], fp32)
            W1b_t = pp.tile([C, O], fp32)
            W1d_t = pp.tile([C, O], fp32)          # W1a - W1b
            W2_t = pp.tile([O, O], fp32)
            b1_t = pp.tile([1, O], fp32)
            b2_t = pp.tile([1, O], fp32)
            ones_col = pp.tile([1, 128], fp32)     # lhsT for bias seeding
            ident = pp.tile([128, 128], fp32)
            idxoff = pp.tile([128, NG * 8], fp32)  # +1-offset global base per group
            Usb = pp.tile([128, NT * 128], f16)    # u' = xloc@(W1a-W1b)+b1, tile-major
            mtall = pp.tile([128, NT * O], fp32)   # per-tile k-sums, out-mm batched
            W1b16 = pp.tile([C, O], f16)
            W1d16 = pp.tile([C, O], f16)
            b1_16 = pp.tile([1, O], f16)
            ones16 = pp.tile([1, 128], f16)

            nc.sync.dma_start(out=xloca[:, :], in_=XLH[:, :])
            nc.sync.dma_start(out=W1a_t[:, :], in_=W1[0:C, :])
            nc.sync.dma_start(out=W1b_t[:, :], in_=W1[C : 2 * C, :])
            nc.sync.dma_start(out=W2_t[:, :], in_=W2[:, :])
            nc.sync.dma_start(out=b1_t[:, :], in_=b1[:, :])
            nc.sync.dma_start(out=b2_t[:, :], in_=b2[:, :])
            for ch in range(N // 2048):
                nc.sync.dma_start(
                    out=XTa[:, ch * 2048 : (ch + 1) * 2048],
                    in_=XTAH[:, ch * 2048 : (ch + 1) * 2048],
                )

            nc.vector.memset(ones_col[:, :], 1.0)
            for g in range(NG):
                nc.vector.memset(idxoff[:, g * 8 : (g + 1) * 8], float(g * GRP + 1))
            make_identity(nc, ident[:, :])
            nc.vector.tensor_tensor(
                out=W1d_t[:, :], in0=W1a_t[:, :], in1=W1b_t[:, :], op=Alu.subtract
            )
            nc.scalar.activation(out=W1b16[:, :], in_=W1b_t[:, :], func=Act.Copy)
            nc.scalar.activation(out=W1d16[:, :], in_=W1d_t[:, :], func=Act.Copy)
            nc.scalar.activation(out=b1_16[:, :], in_=b1_t[:, :], func=Act.Copy)
            nc.vector.memset(ones16[:, :], 1.0)

            # ---- prep pieces (issued AFTER the first two tiles' scans) ----
            def v_chunk(c):
                # V rows [c*4096, (c+1)*4096) = x @ W1b -> DRAM (row-major)
                Vsb = vcp.tile([128, VROWS // 128 * 128], f16, tag="vsb")
                for b in range(VROWS // 128):
                    col = c * (VROWS // 128) + b
                    vp = aux_ps.tile([128, O], fp32, tag="aux")
                    nc.tensor.matmul(
                        out=vp[:, :],
                        lhsT=XTa[0:C, col * 128 : (col + 1) * 128],
                        rhs=W1b16[:, :],
                        start=True,
                        stop=True,
                    )
                    nc.scalar.activation(
                        out=Vsb[:, b * 128 : (b + 1) * 128],
                        in_=vp[:, :],
                        func=Act.Copy,
                    )
                nc.sync.dma_start(
                    out=Vd[c * VROWS : (c + 1) * VROWS, :].rearrange(
                        "(b p) f -> p b f", p=128
                    ),
                    in_=Vsb[:, :].rearrange("p (b f) -> p b f", f=128),
                )

            def u_prep():
                # u' = xloc @ (W1a - W1b) + b1  (tile-major into Usb)
                for t in range(NT):
                    up = aux_ps.tile([128, O], fp32, tag="aux")
                    nc.tensor.matmul(
                        out=up[:, :],
                        lhsT=ones16[:, :],
                        rhs=b1_16[:, :],
                        start=True,
                        stop=False,
                    )
                    nc.tensor.matmul(
                        out=up[:, :],
                        lhsT=xloca[0:C, t * 128 : (t + 1) * 128],
                        rhs=W1d16[:, :],
                        start=False,
                        stop=True,
                    )
                    nc.scalar.activation(
                        out=Usb[:, t * 128 : (t + 1) * 128],
                        in_=up[:, :],
                        func=Act.Copy,
                    )  # Usb is fp16: enables the 2x DVE u-add

            # ---- main stages ----
            ND = NG  # scan groups per row-tile (1024-wide, straight from PSUM)
            def stage1(t, cbs=None):
                # distance matmuls, exact top-16 indices.  (An ACT-staged
                # 2048-wide SBUF scan variant saved ~78us of DVE time but
                # cost ~220us of boundary stalls -- the fp32 PSUM->SBUF ACT
                # copies get no accel and sit in the DVE's feed path.)
                lhsT_t = xloca[:, t * 128 : (t + 1) * 128]
                cand = sb2.tile([128, ND * 8], fp32, tag="cand")
                cidx = sb2.tile([128, ND * 8], u32, tag="cidx")
                for g in range(NG):
                    sp = s_ps.tile([128, GRP], fp32, tag="sp")
                    for q in range(GRP // 512):
                        nc.tensor.matmul(
                            out=sp[:, q * 512 : (q + 1) * 512],
                            lhsT=lhsT_t,
                            rhs=XTa[:, g * GRP + q * 512 : g * GRP + (q + 1) * 512],
                            start=True,
                            stop=True,
                        )
                    # max8 + find_index8 straight from PSUM
                    nc.vector.max(out=cand[:, g * 8 : (g + 1) * 8], in_=sp[:, :])
                    nc.vector.max_index(
                        out=cidx[:, g * 8 : (g + 1) * 8],
                        in_max=cand[:, g * 8 : (g + 1) * 8],
                        in_values=sp[:, :],
                    )

                # phase B: exact top-16 index extraction (64 candidates)
                cidx_f = sb2.tile([128, ND * 8], fp32, tag="cidx_f")
                nc.vector.tensor_copy(out=cidx_f[:, :], in_=cidx[:, :])
                idxp1 = sb2.tile([128, ND * 8], fp32, tag="idxp1")
                nc.vector.tensor_tensor(
                    out=idxp1[:, :], in0=cidx_f[:, :], in1=idxoff[:, :], op=Alu.add
                )
                m1 = sb2.tile([128, 8], fp32, tag="m1")
                nc.vector.max(out=m1[:, :], in_=cand[:, :])
                cand2 = sb2.tile([128, ND * 8], fp32, tag="cand2")
                nc.vector.match_replace(
                    out=cand2[:, :], in_to_replace=m1[:, :], in_values=cand[:, :],
                    imm_value=NEG,
                )
                m2 = sb2.tile([128, 8], fp32, tag="m2")
                nc.vector.max(out=m2[:, :], in_=cand2[:, :])
                # a stride-0 broadcast operand makes DVE is_ge ~7x slower
                # (1.4us vs 0.19us measured); replicate the threshold on the
                # idle ACT engine and compare flat tensors instead
                thr_rep = sb2.tile([128, ND * 8], fp32, tag="thr_rep")
                nc.scalar.activation(
                    out=thr_rep[:, :],
                    in_=m2[:, 7:8].to_broadcast([128, ND * 8]),
                    func=Act.Copy,
                )
                mask = sb2.tile([128, ND * 8], fp32, tag="mask")
                nc.vector.tensor_tensor(
                    out=mask[:, :],
                    in0=cand[:, :],
                    in1=thr_rep[:, :],
                    op=Alu.is_ge,
                )
                midx = sb2.tile([128, ND * 8], fp32, tag="midx")
                nc.vector.tensor_tensor(
                    out=midx[:, :], in0=mask[:, :], in1=idxp1[:, :], op=Alu.mult
                )
                winners = sb2.tile([128, 16], fp32, tag="winners")
                nc.vector.max(out=winners[:, 0:8], in_=midx[:, :])
                midx2 = sb2.tile([128, ND * 8], fp32, tag="midx2")
                nc.vector.match_replace(
                    out=midx2[:, :], in_to_replace=winners[:, 0:8],
                    in_values=midx[:, :], imm_value=0.0,
                )
                nc.vector.max(out=winners[:, 8:16], in_=midx2[:, :])
                nc.vector.tensor_scalar_add(winners[:, :], winners[:, :], -1.0)
                idxu = sb2.tile([128, 16], u32, tag="idxu")
                nc.vector.tensor_copy(out=idxu[:, :], in_=winners[:, :])
                return idxu

            def gather_stage(t, idxu):
                # 16 single-offset indirect DMAs (HW ignores all but the
                # first offset column of a multi-offset AP -- verified on HW)
                Gt = atp.tile([128, K * O], f16, tag="at")
                for k in range(K):
                    nc.gpsimd.indirect_dma_start(
                        out=Gt[:, k * O : (k + 1) * O],
                        out_offset=None,
                        in_=Vd[:, :],
                        in_offset=bass.IndirectOffsetOnAxis(
                            ap=idxu[:, k : k + 1], axis=0
                        ),
                    )
                return Gt

            def stage2(t, Gt):
                # a = G + u' ; h = relu(a) ; m-sum over k via halving tree
                # u-add runs on the DVE: the gpsimd queue saturates with the
                # 16 gathers/tile, sliding stage2 to the tile boundary where
                # the scheduler-placed transposes then stall the PE queue.
                # All-fp16 operands put the add in the 2x_1p DVE mode.
                At = sb.tile([128, K * O], f16, tag="at2")
                nc.vector.tensor_tensor(
                    out=At[:, :].rearrange("p (k f) -> p k f", k=K),
                    in0=Gt[:, :].rearrange("p (k f) -> p k f", k=K),
                    in1=Usb[:, t * 128 : (t + 1) * 128]
                    .rearrange("p (k f) -> p k f", k=1)
                    .to_broadcast([128, K, O]),
                    op=Alu.add,
                )
                Ht = sb.tile([128, K * O], fp32, tag="ht")
                nc.scalar.activation(out=Ht[:, :], in_=At[:, :], func=Act.Relu)
                T1 = sb2.tile([128, 8 * O], fp32, tag="T1")
                nc.gpsimd.tensor_tensor(
                    out=T1[:, :], in0=Ht[:, 0 : 8 * O], in1=Ht[:, 8 * O : 16 * O],
                    op=Alu.add,
                )
                T2 = sb2.tile([128, 4 * O], fp32, tag="T2")
                nc.gpsimd.tensor_tensor(
                    out=T2[:, :], in0=T1[:, 0 : 4 * O], in1=T1[:, 4 * O : 8 * O],
                    op=Alu.add,
                )
                T3 = sb2.tile([128, 2 * O], fp32, tag="T3")
                nc.gpsimd.tensor_tensor(
                    out=T3[:, :], in0=T2[:, 0 : 2 * O], in1=T2[:, 2 * O : 4 * O],
                    op=Alu.add,
                )
                nc.gpsimd.tensor_tensor(
                    out=mtall[:, t * O : (t + 1) * O],
                    in0=T3[:, 0:O], in1=T3[:, O : 2 * O], op=Alu.add,
                )

            def out_stage(t):
                # out_tile = (m/16) @ W2 + b2
                mtp = aux_ps.tile([128, 128], fp32, tag="aux")
                nc.tensor.transpose(
                    out=mtp[:, :], in_=mtall[:, t * O : (t + 1) * O],
                    identity=ident[:, :],
                )
                mT = sb2.tile([128, 128], fp32, tag="mT")
                nc.scalar.activation(
                    out=mT[:, :], in_=mtp[:, :], func=Act.Copy, scale=1.0 / K
                )
                op_ = aux_ps.tile([128, O], fp32, tag="aux")
                nc.tensor.matmul(
                    out=op_[:, :], lhsT=ones_col[:, :], rhs=b2_t[:, :],
                    start=True, stop=False,
                )
                nc.tensor.matmul(
                    out=op_[:, :], lhsT=mT[:, :], rhs=W2_t[:, :],
                    start=False, stop=True,
                )
                osb = sb2.tile([128, O], fp32, tag="osb")
                nc.scalar.activation(out=osb[:, :], in_=op_[:, :], func=Act.Copy)
                nc.sync.dma_start(
                    out=out[t * 128 : (t + 1) * 128, :], in_=osb[:, :]
                )

            # pipeline:
            #   t=0: pure scan (PE starts distance MMs immediately),
            #        then ALL V/u' prep (program order: every Vd write and
            #        the Usb fill must precede the first gather/prefill)
            #   t>=2: gather(t-2) at top of iteration t
            #   stage2(tg) at g==8 once two gathers are in flight
            #   out_stage(t-5) at g==13 (deep delay so the PE queue never
            #   holds a transpose whose gpsimd input chain is still cold)
            # The gpsimd queue is nearly saturated (16 gathers ~22.5us +
            # stage2 ~11us per ~37us scan), so stage2 execution slides to
            # the tile boundaries.  Nothing may couple the PE queue to it:
            # ALL out_stages (transpose + W2 matmuls) are issued after the
            # main loop, where their tree inputs are long complete.
            pend = []      # (t, idxu) awaiting gather issue
            done_g = []    # (t, Gt) awaiting stage2
            out_pend = []  # tiles whose stage2 is issued, awaiting out_stage
            for t in range(NT):
                if len(done_g) >= 2:
                    tg2, Gt2 = done_g.pop(0)
                    stage2(tg2, Gt2)
                    out_pend.append(tg2)
                if t >= 1 and pend:
                    tg, ixu = pend.pop(0)
                    done_g.append((tg, gather_stage(tg, ixu)))
                pend.append((t, stage1(t)))
                if t == 0:
                    u_prep()
                    for c in range(VCHUNKS):
                        v_chunk(c)
            while pend:
                tg, ixu = pend.pop(0)
                done_g.append((tg, gather_stage(tg, ixu)))
            while done_g:
                tg, Gt2 = done_g.pop(0)
                stage2(tg, Gt2)
                out_pend.append(tg)
            while out_pend:
                out_stage(out_pend.pop(0))
    nc.finalize()
    return nc


LAST_RESULTS = None


def kernel(x, W1, b1, W2, b2):
    global LAST_RESULTS
    from concourse.bass_utils import run_bass_kernel_spmd

    if "nc" not in _CACHE:
        _CACHE["nc"] = _build_module()
    nc = _CACHE["nc"]

    x = np.ascontiguousarray(np.asarray(x, dtype=np.float32))
    xh = x.astype(np.float16)
    v = -0.5 * (xh.astype(np.float32) ** 2).sum(axis=1)
    hi = v.astype(np.float16)
    lo = (v - hi.astype(np.float32)).astype(np.float16)
    XTAH = np.zeros((CAUG, N), dtype=np.float16)
    XTAH[0:C, :] = xh.T
    XTAH[C, :] = hi
    XTAH[C + 1, :] = lo
    XLH_full = np.zeros((CAUG, N), dtype=np.float16)
    XLH_full[0:C, :] = xh.T
    XLH_full[C, :] = 1.0
    XLH_full[C + 1, :] = 1.0

    xT = np.ascontiguousarray(x.T)
    in_maps = []
    for c in range(NCORES):
        sl = slice(c * RLOC, (c + 1) * RLOC)
        in_maps.append(
            {
                "XTAH": XTAH,
                "XLH": np.ascontiguousarray(XLH_full[:, sl]),
                "xT": xT,
                "xlocT": np.ascontiguousarray(x[sl, :].T),
                "W1": np.ascontiguousarray(np.asarray(W1, dtype=np.float32)),
                "b1": np.ascontiguousarray(
                    np.asarray(b1, dtype=np.float32).reshape(1, O)
                ),
                "W2": np.ascontiguousarray(np.asarray(W2, dtype=np.float32)),
                "b2": np.ascontiguousarray(
                    np.asarray(b2, dtype=np.float32).reshape(1, O)
                ),
            }
        )
    import os

    res = run_bass_kernel_spmd(
        nc,
        in_maps,
        core_ids=list(range(NCORES)),
        trace=bool(int(os.environ.get("KERNEL_TRACE", "0"))),
    )
    LAST_RESULTS = res
    outs = [res.results[c]["out"] for c in range(NCORES)]
    return np.concatenate(outs, axis=0).astype(np.float32)
